# revision 1
# baseline (speedup 1.0000x reference)
"""Bidirectional LSTM Trainium2 Bass kernel.

Problem: T=128, B=128, IN=512, H=512, OUT=512 (fp32 reference).
Sharding: data-parallel over batch + direction-parallel:
  cores 0-3: forward LSTM, batch slices 0:32, 32:64, 64:96, 96:128
  cores 4-7: backward LSTM (time-reversed x), same batch slices
Each core (phases overlap via dependency scheduling; phase-1/3 work is
emitted interleaved into the recurrence so the in-order TensorE stream
fills recurrence stalls):
  phase 1: xw[t] = x[t] @ W_ih.T + (b_ih + b_hh), 4 timesteps per GEMM
           (M=128); bias added by VectorE during PSUM evacuation into an
           8-chunk SBUF ring consumed by phase 2
  phase 2: 128 sequential LSTM steps:
           gates = xw[t] (seeded into PSUM via a column-selection matmul
           against ident128, which also sets PSUM has_written)
                 + h[t-1] @ W_hh.T (4 K-tile matmuls per 512-col bank)
           sigmoid/tanh on ScalarE, cell update on VectorE,
           h transposed on TensorE for the next step's stationary operand.
           All per-step tensors are split into per-half (256-hidden-unit)
           tiles (gates, acts, c, h, and the hT history split by k-pair)
           because Tile tracks dependencies at tile granularity - the
           split lets each half's chain and the next step's matmuls
           overlap.
  phase 3: partial out[t] = h[t] @ W_lin[:, dir*H:(dir+1)*H].T into an
           SBUF buffer, one final DMA to DRAM
Host combines: out = out_fwd + flip_t(out_bwd) + b_lin.

All matmuls run in bf16 (fp32 PSUM accumulation); the cell state c stays
fp32. Gate columns are host-permuted to [o f i g] per 256-wide half so
one sigmoid instruction covers o,f,i contiguously.
"""

import sys

sys.path.insert(0, "/opt/trn_rl_repo")

import functools

import ml_dtypes
import numpy as np

import concourse.bass as bass
import concourse.tile as tile
from concourse import bacc, mybir
from concourse.bass_utils import run_bass_kernel_spmd

T, B, IN, H, OUT = 128, 128, 512, 512, 512
NCORES = 8
BL = B // 4  # batch per core (4 cores per direction)
G4 = 4 * H  # 2048 gate columns
KT = IN // 128  # 4 K-tiles of 128
NB = G4 // 512  # 4 psum banks of 512 gate cols
TCH = T // 4  # 32 chunks of 4 timesteps for phase 1/3
RING = int(__import__("os").environ.get("LSTM_RING", "8"))  # xw ring depth (chunks)

import os

KNOB_LOOKAHEAD = int(os.environ.get("LSTM_LOOKAHEAD", "4"))
KNOB_XPOSE_DMA = os.environ.get("LSTM_XPOSE_DMA", "0") == "1"
KNOB_MERGE_HALVES = os.environ.get("LSTM_MERGE_HALVES", "0") == "1"
KNOB_SPLIT_FI = os.environ.get("LSTM_SPLIT_FI", "0") == "1"
KNOB_COPIES_ACT = os.environ.get("LSTM_COPIES_ACT", "0") == "1"
KNOB_BUFS_ACTS = int(os.environ.get("LSTM_BUFS_ACTS", "2"))
KNOB_BUFS_TMPS = int(os.environ.get("LSTM_BUFS_TMPS", "2"))
KNOB_PST_BUFS = int(os.environ.get("LSTM_PST_BUFS", "1"))

BF16 = mybir.dt.bfloat16
FP32 = mybir.dt.float32
AF = mybir.ActivationFunctionType


def build_nc(reps=1):
    nc = bacc.Bacc(None, target_bir_lowering=False)
    xT = nc.dram_tensor("xT", [128, TCH, KT, 4, BL], BF16, kind="ExternalInput")
    wih = nc.dram_tensor("wih", [128, KT, G4], BF16, kind="ExternalInput")
    whh = nc.dram_tensor("whh", [128, KT, G4], BF16, kind="ExternalInput")
    bias = nc.dram_tensor("biasr", [128, G4], BF16, kind="ExternalInput")
    wlin = nc.dram_tensor("wlin", [128, KT, OUT], BF16, kind="ExternalInput")
    id32 = nc.dram_tensor("id32", [BL, BL], BF16, kind="ExternalInput")
    id128 = nc.dram_tensor("id128", [128, 128], BF16, kind="ExternalInput")
    outp = nc.dram_tensor("outp", [128, TCH, OUT], FP32, kind="ExternalOutput")

    with tile.TileContext(nc) as tc:
        with (
            tc.tile_pool(name="const", bufs=1) as constp,
            tc.tile_pool(name="xwring", bufs=RING) as ringp,
            tc.tile_pool(name="p1x", bufs=4) as p1x,
            tc.tile_pool(name="acts", bufs=KNOB_BUFS_ACTS) as acts_p,
            tc.tile_pool(name="tmps", bufs=KNOB_BUFS_TMPS) as tmps_p,
            tc.tile_pool(name="p1ps", bufs=1, space="PSUM") as p1ps,
            tc.tile_pool(name="ps2", bufs=1, space="PSUM") as ps2,
            tc.tile_pool(name="psT", bufs=1, space="PSUM") as psT,
            tc.tile_pool(name="ps3", bufs=1, space="PSUM") as ps3,
        ):
            id32_sb = constp.tile([BL, BL], BF16)
            nc.sync.dma_start(id32_sb[:], id32[:])
            id128_sb = constp.tile([128, 128], BF16)
            nc.sync.dma_start(id128_sb[:], id128[:])
            wih_sb = constp.tile([128, KT, G4], BF16)
            nc.sync.dma_start(wih_sb[:], wih[:])
            whh_sb = constp.tile([128, KT, G4], BF16)
            nc.sync.dma_start(whh_sb[:], whh[:])
            bias_sb = constp.tile([128, G4], BF16)
            nc.sync.dma_start(bias_sb[:], bias[:])
            wlin_sb = constp.tile([128, KT, OUT], BF16)
            nc.sync.dma_start(wlin_sb[:], wlin[:])
            # h^T history, split by k-pair so the next step's k=0,1 matmuls
            # depend only on the half-0 copy (Tile tracks deps per tile).
            # hT_k[p][:, ch, kk, ti, :] holds h_t[128*(2p+kk):...,:] bf16.
            hT_k = [
                constp.tile([128, TCH, 2, 4, BL], BF16, name=f"hTk{p}")
                for p in range(2)
            ]
            hT0 = constp.tile([128, KT, BL], BF16)
            nc.vector.memset(hT0[:], 0.0)
            # cell state per half (separate tiles -> independent dep chains)
            c_half = [constp.tile([BL, H // 2], FP32, name=f"c{q}") for q in range(2)]
            out_all = constp.tile([128, TCH, OUT], FP32)

            for _rep in range(reps):
                for q in range(2):
                    nc.vector.memset(c_half[q][:], 0.0)
                xw_tiles = []
                n_halves_emitted = [0]

                # ---- phase 1 emitter: xw = x @ W_ih.T + bias, emitted in
                # half-chunk (10-matmul) granules spread across the
                # recurrence so the scheduler can fill PE stalls.
                def emit_xw_half():
                    hidx = n_halves_emitted[0]
                    if hidx >= 2 * TCH:
                        return
                    n_halves_emitted[0] += 1
                    ch, half = hidx // 2, hidx % 2
                    if half == 0:
                        xt = p1x.tile([128, KT, 4, BL], BF16, tag="xt")
                        nc.sync.dma_start(xt[:], xT[:, ch])
                        xwr = ringp.tile([128, G4], BF16, tag="xw")
                        xw_tiles.append((xwr, xt))
                    xwr, xt = xw_tiles[ch]
                    for nb2 in range(2):
                        pxw = p1ps.tile([128, 512], FP32, tag="pxw")
                        cg = slice(
                            1024 * half + 512 * nb2, 1024 * half + 512 * nb2 + 512
                        )
                        for k in range(KT):
                            nc.tensor.matmul(
                                pxw[:],
                                xt[:, k],
                                wih_sb[:, k, cg],
                                start=(k == 0),
                                stop=(k == KT - 1),
                            )
                        nc.vector.tensor_add(xwr[:, cg], pxw[:], bias_sb[:, cg])

                # ---- phase 3 emitter: partial linear out = h @ W_lin_half.T
                def emit_linear_chunk(ch):
                    po = ps3.tile([128, OUT], FP32, tag="po")
                    for k in range(KT):
                        nc.tensor.matmul(
                            po[:],
                            hT_k[k // 2][:, ch, k % 2],
                            wlin_sb[:, k],
                            start=(k == 0),
                            stop=(k == KT - 1),
                        )
                    nc.vector.tensor_copy(out_all[:, ch, :], po[:])

                def emit_seeds(t):
                    # seed PSUM with xw row-block (sets has_written);
                    # one psum tile PER BANK -> finest dep granularity
                    ch, ti = t // 4, t % 4
                    gh = [
                        ps2.tile([BL, 512], FP32, tag=f"gates{b}", name=f"gates{b}")
                        for b in range(4)
                    ]
                    for b in range(4):
                        nc.tensor.matmul(
                            gh[b][:],
                            id128_sb[:, 32 * ti : 32 * ti + 32],
                            xw_tiles[ch][0][:, 512 * b : 512 * b + 512],
                            start=True,
                            stop=False,
                        )
                    return gh

                # prologue: phase-1 lookahead + step 0 seeds
                for _ in range(2 * KNOB_LOOKAHEAD):
                    emit_xw_half()
                gh = emit_seeds(0)

                for t in range(T):
                    ch, ti = t // 4, t % 4
                    # W-MMs: half-0's 8 matmuls first (its activations can
                    # then start mid-block); k ascending within each half so
                    # the previous step's half-1 transpose has slack.
                    for q in range(2):
                        for k in range(KT):
                            hT_prev = (
                                hT0[:, k]
                                if t == 0
                                else hT_k[k // 2][
                                    :, (t - 1) // 4, k % 2, (t - 1) % 4, :
                                ]
                            )
                            for nb2 in range(2):
                                nc.tensor.matmul(
                                    gh[2 * q + nb2][:],
                                    hT_prev,
                                    whh_sb[:, k, 1024 * q + 512 * nb2 :
                                           1024 * q + 512 * nb2 + 512],
                                    start=False,
                                    stop=(k == KT - 1),
                                )
                    ah = [
                        acts_p.tile([BL, 1024], BF16, tag=f"acts{q}", name=f"acts{q}")
                        for q in range(2)
                    ]
                    tct = [
                        tmps_p.tile([BL, 256], BF16, tag=f"tct{q}", name=f"tct{q}")
                        for q in range(2)
                    ]
                    fc = [
                        tmps_p.tile([BL, 256], FP32, tag=f"fc{q}", name=f"fc{q}")
                        for q in range(2)
                    ]
                    ig = [
                        tmps_p.tile([BL, 256], FP32, tag=f"ig{q}", name=f"ig{q}")
                        for q in range(2)
                    ]
                    h_sb = [
                        tmps_p.tile([BL, 256], BF16, tag=f"hsb{q}", name=f"hsb{q}")
                        for q in range(2)
                    ]
                    hTp = [
                        psT.tile([128, 2, BL], BF16, tag=f"hTp{q}", name=f"hTp{q}")
                        for q in range(2)
                    ]
                    # ACT: gate activations for both halves first, split
                    # per gate bank so each releases its PSUM tile early
                    for q in range(2):
                        nc.scalar.activation(
                            ah[q][:, 0:512], gh[2 * q][:], AF.Sigmoid
                        )
                        nc.scalar.activation(
                            ah[q][:, 512:768], gh[2 * q + 1][:, 0:256], AF.Sigmoid
                        )
                        nc.scalar.activation(
                            ah[q][:, 768:1024], gh[2 * q + 1][:, 256:512], AF.Tanh
                        )
                    # DVE: c updates for both halves
                    for q in range(2):
                        nc.vector.tensor_mul(
                            fc[q][:], ah[q][:, 256:512], c_half[q][:]
                        )
                        nc.vector.tensor_mul(
                            ig[q][:], ah[q][:, 512:768], ah[q][:, 768:1024]
                        )
                        nc.vector.tensor_add(c_half[q][:], fc[q][:], ig[q][:])
                    # PE stream: fill work before next seeds (in-order engine)
                    if t % 2 == 0:
                        emit_xw_half()
                    if ti == 2 and ch > 0:
                        emit_linear_chunk(ch - 1)
                    if t + 1 < T:
                        gh_next = emit_seeds(t + 1)
                    # tail per half: tanh(c) -> h -> transpose -> hT copy
                    for q in range(2):
                        nc.scalar.activation(tct[q][:], c_half[q][:], AF.Tanh)
                        nc.vector.tensor_mul(h_sb[q][:], ah[q][:, 0:256], tct[q][:])
                        for kk in range(2):
                            nc.tensor.transpose(
                                hTp[q][:, kk],
                                h_sb[q][:, 128 * kk : 128 * kk + 128],
                                id32_sb[:],
                            )
                        nc.vector.tensor_copy(hT_k[q][:, ch, :, ti, :], hTp[q][:])
                    if t + 1 < T:
                        gh = gh_next

                emit_linear_chunk(TCH - 1)
            nc.sync.dma_start(outp[:], out_all[:])
    nc.compile()
    return nc


@functools.lru_cache(maxsize=1)
def _program():
    return build_nc()


def _gate_perm():
    # PyTorch gate row order: i (0:H), f (H:2H), g (2H:3H), o (3H:4H).
    # Target layout per 1024-col half q: [o f i g] x 256 covering hidden
    # units 256q:256q+256, so sigmoid spans 768 contiguous cols.
    perm = []
    for q in range(2):
        perm += list(range(3 * H + 256 * q, 3 * H + 256 * q + 256))  # o
        perm += list(range(1 * H + 256 * q, 1 * H + 256 * q + 256))  # f
        perm += list(range(0 * H + 256 * q, 0 * H + 256 * q + 256))  # i
        perm += list(range(2 * H + 256 * q, 2 * H + 256 * q + 256))  # g
    return np.asarray(perm)


def _prep_core(x, W_ih, W_hh, b_ih, b_hh, W_lin, direction, bs):
    perm = _gate_perm()
    bf16 = ml_dtypes.bfloat16
    xs = np.asarray(x)[:, bs : bs + BL, :]
    if direction == 1:
        xs = xs[::-1]
    # xT[p, ch, k, ti, b] = xs[4*ch+ti, b, 128k+p]
    xT = np.ascontiguousarray(
        xs.reshape(TCH, 4, BL, KT, 128).transpose(4, 0, 3, 1, 2)
    ).astype(bf16)
    Wp_ih = np.asarray(W_ih)[perm]  # [G4, IN]
    Wp_hh = np.asarray(W_hh)[perm]
    bp = (np.asarray(b_ih) + np.asarray(b_hh))[perm].astype(np.float32)
    wih = np.ascontiguousarray(Wp_ih.T.reshape(KT, 128, G4).transpose(1, 0, 2)).astype(
        bf16
    )
    whh = np.ascontiguousarray(Wp_hh.T.reshape(KT, 128, G4).transpose(1, 0, 2)).astype(
        bf16
    )
    biasr = np.ascontiguousarray(
        np.broadcast_to(bp.astype(bf16), (128, G4))
    )
    ones = np.ones((1, 128), dtype=bf16)
    Wl = np.asarray(W_lin)[:, direction * H : (direction + 1) * H]  # [OUT, H]
    wlin = np.ascontiguousarray(Wl.T.reshape(KT, 128, OUT).transpose(1, 0, 2)).astype(
        bf16
    )
    return {
        "xT": xT,
        "wih": wih,
        "whh": whh,
        "biasr": biasr,
        "ones": ones,
        "wlin": wlin,
        "id32": np.eye(BL, dtype=bf16),
        "id128": np.eye(128, dtype=bf16),
    }


def run_cores(inputs, trace=False):
    """Build per-core in_maps, run on 8 cores, return BassKernelResults."""
    in_maps = []
    for core in range(NCORES):
        direction = core // 4
        bs = (core % 4) * BL
        wk = "f" if direction == 0 else "b"
        in_maps.append(
            _prep_core(
                inputs["x"],
                inputs[f"W_ih_{wk}"],
                inputs[f"W_hh_{wk}"],
                inputs[f"b_ih_{wk}"],
                inputs[f"b_hh_{wk}"],
                inputs["W_lin"],
                direction,
                bs,
            )
        )
    nc = _program()
    return run_bass_kernel_spmd(nc, in_maps, list(range(NCORES)), trace=trace)


def _assemble(results, b_lin):
    # per-core outp: [128(=4 ti x 32 b), TCH, OUT] in compute-time order
    out = np.zeros((T, B, OUT), np.float32)
    for core in range(NCORES):
        direction = core // 4
        bs = (core % 4) * BL
        dev = np.asarray(results[core]["outp"], np.float32)  # [128, TCH, OUT]
        # t = 4*ch + ti, partition p = 32*ti + b
        part = (
            dev.reshape(4, BL, TCH, OUT).transpose(2, 0, 1, 3).reshape(T, BL, OUT)
        )
        if direction == 1:
            part = part[::-1]
        out[:, bs : bs + BL, :] += part
    out += np.asarray(b_lin, np.float32)[None, None, :]
    return out


def kernel(**inputs):
    res = run_cores(inputs, trace=False)
    return _assemble(res.results, inputs["b_lin"])



# revision 2
# speedup vs baseline: 1.7867x; 1.7867x over previous
"""Bidirectional LSTM Trainium2 Bass kernel (transposed formulation).

Problem: T=128, B=128, IN=512, H=512, OUT=512 (fp32 reference).
Sharding: data-parallel over batch + direction-parallel:
  cores 0-3: forward LSTM, batch slices 0:32, 32:64, 64:96, 96:128
  cores 4-7: backward LSTM (time-reversed x), same batch slices

Transposed layout: gates live on the PARTITION axis (16 stationary
chunks of 128 gates = (type o/f/i/g) x (hidden 128-chunk)), the batch
(32) is the matmul moving dim.  PE matmul cost scales with the moving
free size only, so this quarters TensorE work vs. batch-in-partition:
  - phase 1 (xw = x @ W_ih.T + bias) accumulates straight into the
    per-step PSUM bank: bias seeded by a K=1 ones-matmul (start=True),
    then 4 x-k-tile matmuls, then at step time 4 W_hh k-tile matmuls.
  - per-step PSUM bank: tile [128, 4(type), 4(hid chunk), 32] fp32
    (= 512 cols = exactly one 2KB PSUM bank), 6 banks in flight.
  - h is produced directly in transposed layout [hid-in-chunk(128),
    chunk, t, batch] -> no transpose instructions at all; the next
    step's matmuls and phase 3 read it as the moving operand.
  - activations: sigmoid over (o,f,i) block slices, tanh over g;
    cell update on DVE; h = o*tanh(c) on GPSIMD (idle engine) so the
    DVE queue is never blocked behind ACT.
  - phase 3 (out = h @ W_lin_dir.T) per 4-step granule into 1 PSUM
    bank (double buffered), evacuated by GPSIMD, DMA'd per granule.
Host combines: out = out_fwd + flip_t(out_bwd) + b_lin.

All matmuls bf16 (fp32 PSUM accumulation); cell state stays fp32.
"""

import sys

sys.path.insert(0, "/opt/trn_rl_repo")

import functools
import os

import ml_dtypes
import numpy as np

import concourse.bass as bass
import concourse.tile as tile
from concourse import bacc, mybir
from concourse.bass_utils import run_bass_kernel_spmd

T, B, IN, H, OUT = 128, 128, 512, 512, 512
NCORES = 8
BL = B // 4  # batch per core (4 cores per direction)
G4 = 4 * H  # 2048 gate columns
KT = IN // 128  # 4 k-tiles of 128
NCH = 16  # gate M-chunks: (type o/f/i/g) x (hidden chunk 0..3)
OCH = OUT // 128  # 4 output column chunks
TCH = T // 4  # 32 output granules of 4 timesteps

KNOB_LOOKAHEAD = int(os.environ.get("LSTM_LOOKAHEAD", "4"))
KNOB_PG_BUFS = int(os.environ.get("LSTM_PG_BUFS", "6"))

BF16 = mybir.dt.bfloat16
FP32 = mybir.dt.float32
AF = mybir.ActivationFunctionType


def build_nc(reps=1):
    nc = bacc.Bacc(None, target_bir_lowering=False)
    xT = nc.dram_tensor("xT", [128, KT, T, BL], BF16, kind="ExternalInput")
    wih = nc.dram_tensor("wih", [128, KT, G4], BF16, kind="ExternalInput")
    whh = nc.dram_tensor("whh", [128, KT, G4], BF16, kind="ExternalInput")
    bias1 = nc.dram_tensor("bias1", [1, G4], BF16, kind="ExternalInput")
    ones1 = nc.dram_tensor("ones1", [1, BL], BF16, kind="ExternalInput")
    wlin = nc.dram_tensor("wlin", [128, KT, OUT], BF16, kind="ExternalInput")
    outp = nc.dram_tensor("outp", [128, OCH, T, BL], FP32, kind="ExternalOutput")

    LA = KNOB_LOOKAHEAD

    with tile.TileContext(nc) as tc:
        with (
            tc.tile_pool(name="const", bufs=1) as constp,
            tc.tile_pool(name="acts", bufs=2) as acts_p,
            tc.tile_pool(name="tmps", bufs=2) as tmps_p,
            tc.tile_pool(name="stag", bufs=2) as stag_p,
            tc.tile_pool(name="pg", bufs=KNOB_PG_BUFS, space="PSUM") as pg_p,
            tc.tile_pool(name="ps3", bufs=2, space="PSUM") as ps3_p,
        ):
            wih_sb = constp.tile([128, KT, G4], BF16)
            nc.sync.dma_start(wih_sb[:], wih[:])
            bias_sb = constp.tile([1, G4], BF16)
            nc.sync.dma_start(bias_sb[:], bias1[:])
            ones_sb = constp.tile([1, BL], BF16)
            nc.sync.dma_start(ones_sb[:], ones1[:])
            # x in 4 time-quarters so phase 1 can start after the first
            x_sb = constp.tile([128, KT, T, BL], BF16)
            for q in range(4):
                nc.sync.dma_start(
                    x_sb[:, :, 32 * q : 32 * q + 32, :], xT[:, :, 32 * q : 32 * q + 32, :]
                )
            whh_sb = constp.tile([128, KT, G4], BF16)
            nc.sync.dma_start(whh_sb[:], whh[:])
            wlin_sb = constp.tile([128, KT, OUT], BF16)
            nc.sync.dma_start(wlin_sb[:], wlin[:])

            # h history, transposed: hh?[p, c, t+1, b] = h_t[128*(2?+c)+p, b]
            # split into per-half tiles so step t+1's k=0,1 matmuls depend
            # only on the A-half write.
            hhA = constp.tile([128, 2, T + 1, BL], BF16)
            hhB = constp.tile([128, 2, T + 1, BL], BF16)
            # cell state [p, hid chunk, b], fp32
            c_st = constp.tile([128, KT, BL], FP32)

            for _rep in range(reps):
                nc.vector.memset(c_st[:], 0.0)
                nc.vector.memset(hhA[:, :, 0, :], 0.0)
                nc.vector.memset(hhB[:, :, 0, :], 0.0)

                pg_tiles = {}

                def emit_phase1(s):
                    # bias seed (K=1 ones matmul, start=True) + x @ W_ih.T
                    pg = pg_p.tile([128, 4, KT, BL], FP32, tag="pg", name=f"pg{s}")
                    pg_tiles[s] = pg
                    for m in range(NCH):
                        ty, hc = m // 4, m % 4
                        nc.tensor.matmul(
                            pg[:, ty, hc],
                            bias_sb[:, 128 * m : 128 * m + 128],
                            ones_sb[:],
                            start=True,
                            stop=False,
                        )
                    for k in range(KT):
                        for m in range(NCH):
                            ty, hc = m // 4, m % 4
                            nc.tensor.matmul(
                                pg[:, ty, hc],
                                wih_sb[:, k, 128 * m : 128 * m + 128],
                                x_sb[:, k, s, :],
                                start=False,
                                stop=False,
                            )

                def emit_phase3(g):
                    # out granule: steps 4g..4g+3 (hh slots 4g+1..4g+4)
                    po = ps3_p.tile([128, OCH, 4, BL], FP32, tag="po", name=f"po{g}")
                    for oc in range(OCH):
                        for k in range(KT):
                            hh = (hhA, hhB)[k // 2]
                            nc.tensor.matmul(
                                po[:, oc],
                                wlin_sb[:, k, 128 * oc : 128 * oc + 128],
                                hh[:, k % 2, 4 * g + 1 : 4 * g + 5, :],
                                start=(k == 0),
                                stop=(k == KT - 1),
                            )
                    st = stag_p.tile([128, OCH, 4, BL], FP32, tag="st", name=f"st{g}")
                    nc.gpsimd.tensor_copy(st[:], po[:])
                    nc.sync.dma_start(outp[:, :, 4 * g : 4 * g + 4, :], st[:])

                for s in range(LA):
                    emit_phase1(s)

                for t in range(T):
                    pg = pg_tiles.pop(t)
                    # W_hh matmuls, k-major so k=0,1 (needing only the
                    # A-half of h(t-1)) issue while the B-half finishes.
                    # Within each k: A-half gate chunks first.
                    for k in range(KT):
                        hh = (hhA, hhB)[k // 2]
                        rhs = hh[:, k % 2, t, :]
                        for m in (0, 1, 4, 5, 8, 9, 12, 13, 2, 3, 6, 7, 10, 11, 14, 15):
                            ty, hc = m // 4, m % 4
                            nc.tensor.matmul(
                                pg[:, ty, hc],
                                whh_sb[:, k, 128 * m : 128 * m + 128],
                                rhs,
                                start=False,
                                stop=(k == KT - 1),
                            )
                    # phase 3 for the completed granule (reads hh slots <= t)
                    if t % 4 == 0 and t > 0:
                        emit_phase3(t // 4 - 1)

                    acts = acts_p.tile([128, 3, KT, BL], BF16, tag="acts", name="acts")
                    gact = tmps_p.tile([128, KT, BL], BF16, tag="gact", name="gact")
                    fc = tmps_p.tile([128, KT, BL], FP32, tag="fc", name="fc")
                    ig = tmps_p.tile([128, KT, BL], FP32, tag="ig", name="ig")
                    tct = tmps_p.tile([128, KT, BL], BF16, tag="tct", name="tct")
                    # ACT queue: sigA tanhA sigB tanhB tctA tctB
                    for h2 in range(2):
                        cs = slice(2 * h2, 2 * h2 + 2)
                        nc.scalar.activation(
                            acts[:, :, cs, :], pg[:, 0:3, cs, :], AF.Sigmoid
                        )
                        nc.scalar.activation(gact[:, cs, :], pg[:, 3, cs, :], AF.Tanh)
                    for h2 in range(2):
                        cs = slice(2 * h2, 2 * h2 + 2)
                        # DVE: c = f*c + i*g
                        nc.vector.tensor_mul(fc[:, cs, :], acts[:, 1, cs, :], c_st[:, cs, :])
                        nc.vector.tensor_mul(ig[:, cs, :], acts[:, 2, cs, :], gact[:, cs, :])
                        nc.vector.tensor_add(c_st[:, cs, :], fc[:, cs, :], ig[:, cs, :])
                        # ACT: tanh(c); GPSIMD: h = o * tanh(c) -> hh slot t+1
                        nc.scalar.activation(tct[:, cs, :], c_st[:, cs, :], AF.Tanh)
                        hh = (hhA, hhB)[h2]
                        nc.gpsimd.tensor_mul(
                            hh[:, :, t + 1, :], acts[:, 0, cs, :], tct[:, cs, :]
                        )
                    if t + LA < T:
                        emit_phase1(t + LA)
                emit_phase3(TCH - 1)
    nc.compile()
    return nc


@functools.lru_cache(maxsize=1)
def _program():
    return build_nc()


def _gate_perm():
    # PyTorch gate row order: i (0:H), f (H:2H), g (2H:3H), o (3H:4H).
    # Target: 16 chunks of 128, chunk m=(type, hc) with type order
    # [o f i g]; within a type block the hidden units are in natural
    # order (hc-major, 128 each).
    off = {0: 3 * H, 1: 1 * H, 2: 0 * H, 3: 2 * H}  # o, f, i, g
    perm = []
    for m in range(NCH):
        ty, hc = m // 4, m % 4
        perm += list(range(off[ty] + 128 * hc, off[ty] + 128 * hc + 128))
    return np.asarray(perm)


def _prep_core(x, W_ih, W_hh, b_ih, b_hh, W_lin, direction, bs):
    perm = _gate_perm()
    bf16 = ml_dtypes.bfloat16
    xs = np.asarray(x)[:, bs : bs + BL, :]
    if direction == 1:
        xs = xs[::-1]
    # xT[p, k, t, b] = xs[t, b, 128k+p]
    xT = np.ascontiguousarray(
        xs.reshape(T, BL, KT, 128).transpose(3, 2, 0, 1)
    ).astype(bf16)
    Wp_ih = np.asarray(W_ih)[perm]  # [G4, IN]
    Wp_hh = np.asarray(W_hh)[perm]
    bp = (np.asarray(b_ih) + np.asarray(b_hh))[perm].astype(np.float32)
    wih = np.ascontiguousarray(Wp_ih.T.reshape(KT, 128, G4).transpose(1, 0, 2)).astype(
        bf16
    )
    whh = np.ascontiguousarray(Wp_hh.T.reshape(KT, 128, G4).transpose(1, 0, 2)).astype(
        bf16
    )
    Wl = np.asarray(W_lin)[:, direction * H : (direction + 1) * H]  # [OUT, H]
    wlin = np.ascontiguousarray(Wl.T.reshape(KT, 128, OUT).transpose(1, 0, 2)).astype(
        bf16
    )
    return {
        "xT": xT,
        "wih": wih,
        "whh": whh,
        "bias1": bp.astype(bf16).reshape(1, G4),
        "ones1": np.ones((1, BL), dtype=bf16),
        "wlin": wlin,
    }


def run_cores(inputs, trace=False):
    """Build per-core in_maps, run on 8 cores, return BassKernelResults."""
    in_maps = []
    for core in range(NCORES):
        direction = core // 4
        bs = (core % 4) * BL
        wk = "f" if direction == 0 else "b"
        in_maps.append(
            _prep_core(
                inputs["x"],
                inputs[f"W_ih_{wk}"],
                inputs[f"W_hh_{wk}"],
                inputs[f"b_ih_{wk}"],
                inputs[f"b_hh_{wk}"],
                inputs["W_lin"],
                direction,
                bs,
            )
        )
    nc = _program()
    return run_bass_kernel_spmd(nc, in_maps, list(range(NCORES)), trace=trace)


def _assemble(results, b_lin):
    # per-core outp: [128(p), OCH, T, BL]; out[t, b, 128*oc+p] = outp[p, oc, t, b]
    out = np.zeros((T, B, OUT), np.float32)
    for core in range(NCORES):
        direction = core // 4
        bs = (core % 4) * BL
        dev = np.asarray(results[core]["outp"], np.float32)  # [128, OCH, T, BL]
        part = dev.transpose(2, 3, 1, 0).reshape(T, BL, OUT)
        if direction == 1:
            part = part[::-1]
        out[:, bs : bs + BL, :] += part
    out += np.asarray(b_lin, np.float32)[None, None, :]
    return out


def kernel(**inputs):
    res = run_cores(inputs, trace=False)
    return _assemble(res.results, inputs["b_lin"])
